# Initial kernel scaffold
#
"""Trainium2 Bass kernel for nn_ConvPair (pairwise-MLP message passing).

Reference computation (N=1024 atoms, F=8 feats, H=128 hidden, O=3 out):
    hi = x @ W1[:F];  hj = x @ W1[F:]
    h  = tanh(hi[:,None,:] + hj[None,:,:] + b1)        # [N,N,H]
    h  = tanh(h @ W2 + b2)                             # [N,N,H]
    y  = tanh(h @ W3 + b3)                             # [N,N,O]
    out = y.sum(axis=(1,2))                            # [N]

Sharding: outer atom dim i split across 8 cores (128 i per core); the small
weights and the full atom table are replicated. No cross-core reduction.

Per-core device pipeline, all tensors hidden-major [H=128 partitions, ...]:
  tanh1: ACT  tanh(HJ_T + hib_i)        one instr per i, bias = HIB col
  mm1:   PE   W2.T-contract             2 matmuls (N=512 each) -> PSUM
  tanh2: ACT  tanh(psum + b2) -> SBUF   bias = b2 column
  mm3:   PE   8x (h2_chunk.T @ W3pad)   pairs-on-partitions -> PSUM [128,32]
  tanh3: ACT  tanh(psum), accum_out     per-channel accumulator -> ACC[:,i]
  final: PE   ACC.T @ ones  ->  per-i scalars, ACT copy, DMA out.

b1 is folded into hib host-side; b2 via the ACT bias port; b3 is zeros for
this problem (asserted; a numpy fallback handles the hypothetical nonzero
case exactly).

Wait-discipline: walrus's Activation codegen supports only one semaphore
wait per instruction, so all constants arrive in ONE DMA and each engine
"touches" that DMA's semaphore once in a warmup instruction; afterwards the
steady-state loop only ever ping-pongs ACT<->PE (one foreign sem each).
"""

import json

import numpy as np
from contextlib import ExitStack

import bass_rust
import concourse.bass as bass
import concourse.tile as tile
from concourse import mybir
from concourse.bass_utils import run_bass_kernel_spmd

f32 = mybir.dt.float32
Tanh = mybir.ActivationFunctionType.Tanh

N, F, H, O = 1024, 8, 128, 3
NCORES = 8
IPC = N // NCORES  # 128 atoms (i) per core
NJ = N             # full j dimension on every core
MM_N = 512         # fp32 matmul max moving free dim
OPAD = 4           # W3 padded 3 -> 4 cols (aligned psum writes; pad col = 0)


def _layout(ipc, nj):
    """Column offsets of the packed constant block [H, ncols]."""
    hj = 0
    hib = hj + nj
    w2 = hib + ipc
    b2 = w2 + H
    w3 = b2 + 1
    ones = w3 + OPAD
    ncols = ones + 1
    return dict(hj=hj, hib=hib, w2=w2, b2=b2, w3=w3, ones=ones, ncols=ncols)


# TPB instructions have a single 8-byte events field: 2 sync commands max
# (walrus rejects more).  Queue-engine DMA ops handle their own sync.
_MULTIWAIT_OK = {"DMACopy", "TriggeredCopy", "Call", "ISA"}


def _legalize_waits(nc):
    """Hoist excess semaphore waits from datapath instructions onto chained
    NoOps (one wait each) so every instruction fits walrus's sync budget."""
    j = json.loads(bass_rust.module_to_json_string(nc.m))
    counter = [0]

    def fix_list(insts):
        out = []
        for inst in insts:
            si = inst.get("sync_info")
            waits = (si or {}).get("on_wait", [])
            if si and len(waits) > 1 and inst.get("opcode") not in _MULTIWAIT_OK:
                # keep zero waits on the instruction; one NoOp per wait
                for w in waits:
                    counter[0] += 1
                    out.append({
                        "debug": inst.get("debug", 0),
                        "engine": inst["engine"],
                        "ins": [],
                        "outs": [],
                        "name": f"W-hoist-{counter[0]}",
                        "opcode": "NoOp",
                        "sync_info": {"on_update": [], "on_wait": [w]},
                    })
                si["on_wait"] = []
            out.append(inst)
        return out

    def walk(o):
        if isinstance(o, dict):
            if "instructions" in o and isinstance(o["instructions"], list):
                o["instructions"] = fix_list(o["instructions"])
            for v in o.values():
                walk(v)
        elif isinstance(o, list):
            for v in o:
                walk(v)

    walk(j)
    nc.m = bass_rust.module_from_json_string(json.dumps(j))
    return counter[0]


def _build(ipc, nj, reps=1):
    """Build the per-core Bass program (SPMD: same program, per-core data).

    reps > 1 repeats the main i-loop (recomputing identical results) and is
    used only for differential timing; outputs are unchanged."""
    assert nj % MM_N == 0 and nj % H == 0
    nchunk = nj // H  # stage-3 chunks of 128 pairs
    lay = _layout(ipc, nj)

    nc = bass.Bass()
    cparam = nc.declare_dram_parameter("c", [H, lay["ncols"]], f32, isOutput=False)
    yparam = nc.declare_dram_parameter("y", [ipc, 1], f32, isOutput=True)

    with tile.TileContext(nc) as tc:
        with ExitStack() as ctx:
            consts = ctx.enter_context(tc.tile_pool(name="consts", bufs=1))
            h1p = ctx.enter_context(tc.tile_pool(name="h1p", bufs=3))
            h2p = ctx.enter_context(tc.tile_pool(name="h2p", bufs=3))
            scrp = ctx.enter_context(tc.tile_pool(name="scrp", bufs=1))
            accp = ctx.enter_context(tc.tile_pool(name="accp", bufs=1))
            # PSUM budget (8 banks): ps1 double-buffer 2x2 + ps3 2x1 + warm 1 + fin 1
            psA = ctx.enter_context(tc.tile_pool(name="psA", bufs=2, space="PSUM"))
            psB = ctx.enter_context(tc.tile_pool(name="psB", bufs=2, space="PSUM"))
            psW = ctx.enter_context(tc.tile_pool(name="psW", bufs=1, space="PSUM"))
            psF = ctx.enter_context(tc.tile_pool(name="psF", bufs=1, space="PSUM"))

            C = consts.tile([H, lay["ncols"]], f32)
            nc.sync.dma_start(out=C, in_=cparam[:, :])

            HJ = C[:, lay["hj"]:lay["hj"] + nj]
            W2 = C[:, lay["w2"]:lay["w2"] + H]
            B2 = C[:, lay["b2"]:lay["b2"] + 1]
            W3 = C[:, lay["w3"]:lay["w3"] + OPAD]
            ONES = C[:, lay["ones"]:lay["ones"] + 1]

            ACC = accp.tile([H, ipc], f32)          # [j-offset, i] partial sums
            warm = scrp.tile([H, 1], f32, tag="warm")

            # --- warmups: let ACT and PE observe the const-DMA semaphore
            # (and load the tanh table) on single-wait instructions.
            nc.scalar.activation(out=warm, in_=B2, func=Tanh)
            warm_ps = psW.tile([1, 1], f32)
            nc.tensor.matmul(warm_ps, C[:, lay["w2"]:lay["w2"] + 1],
                             C[:, lay["w2"]:lay["w2"] + 1], start=True, stop=True)

            # --- main loop: groups of G atoms; tanh1/tanh3 batched per group
            G = 8 if ipc % 8 == 0 else (4 if ipc % 4 == 0 else 1)
            for g in [g for _ in range(reps) for g in range(ipc // G)]:
                # DVE broadcast-adds HJ + hib_i into a [128, G*nj] block,
                # then ONE big ACT tanh covers the whole group.
                h1 = h1p.tile([H, G, nj], f32)
                for k in range(G):
                    i = g * G + k
                    nc.vector.tensor_scalar_add(
                        h1[:, k, :], HJ,
                        C[:, lay["hib"] + i:lay["hib"] + i + 1],
                    )
                nc.scalar.activation(out=h1[:, :, :], in_=h1[:, :, :], func=Tanh)

                ps3 = psB.tile([H, G, nchunk, OPAD], f32, tag="s3")
                for k in range(G):
                    ps1 = psA.tile([H, nj], f32)
                    for t in range(nj // MM_N):
                        nc.tensor.matmul(
                            ps1[:, t * MM_N:(t + 1) * MM_N],
                            W2,
                            h1[:, k, t * MM_N:(t + 1) * MM_N],
                            start=True, stop=True,
                        )
                    h2 = h2p.tile([H, nj], f32)
                    nc.scalar.activation(out=h2, in_=ps1, func=Tanh, bias=B2)
                    for cch in range(nchunk):
                        nc.tensor.matmul(
                            ps3[:, k, cch, :],
                            h2[:, cch * H:(cch + 1) * H],
                            W3,
                            start=True, stop=True,
                        )
                # one in-place tanh over the whole group's [128, G*32] block,
                # then DVE free-axis reduce into ACC columns
                nc.scalar.activation(out=ps3[:, :, :, :], in_=ps3[:, :, :, :],
                                     func=Tanh)
                nc.vector.tensor_reduce(
                    out=ACC[:, g * G:(g + 1) * G],
                    in_=ps3.rearrange("p g c o -> p g (c o)"),
                    axis=mybir.AxisListType.X,
                    op=mybir.AluOpType.add,
                )

            # --- reduce over the 128 j-offset partitions: out = ACC.T @ ones
            fin = psF.tile([ipc, 1], f32)
            nc.tensor.matmul(fin, ACC, ONES, start=True, stop=True)
            yout = scrp.tile([ipc, 1], f32, tag="yout")
            nc.scalar.copy(yout, fin)
            nc.sync.dma_start(out=yparam[:, :], in_=yout)

    _legalize_waits(nc)
    return nc


_NC_CACHE = {}


def _get_nc(ipc, nj):
    key = (ipc, nj)
    if key not in _NC_CACHE:
        _NC_CACHE[key] = _build(ipc, nj)
    return _NC_CACHE[key]


def _host_prep(x, W1, b1, ipc, nj):
    """Build the per-core packed const blocks. Returns list of [H,ncols] f32."""
    lay = _layout(ipc, nj)
    hi = x @ W1[:F]          # [N, H]
    hj = x @ W1[F:]          # [N, H]
    hib = hi + b1[None, :]   # fold b1
    hj_t = np.ascontiguousarray(hj[:nj].T)    # [H, nj]
    return lay, hib, hj_t


def kernel(x, W1, b1, W2, b2, W3, b3):
    x = np.asarray(x, np.float32)
    W1 = np.asarray(W1, np.float32)
    b1 = np.asarray(b1, np.float32)
    W2 = np.asarray(W2, np.float32)
    b2 = np.asarray(b2, np.float32)
    W3 = np.asarray(W3, np.float32)
    b3 = np.asarray(b3, np.float32)

    if np.any(b3 != 0.0):
        # Never hit for this problem (spec fills b3 with zeros); exact
        # numpy fallback keeps the kernel correct for arbitrary inputs.
        return _numpy_ref(x, W1, b1, W2, b2, W3, b3)

    lay, hib, hj_t = _host_prep(x, W1, b1, IPC, NJ)
    W3pad = np.zeros((H, OPAD), np.float32)
    W3pad[:, :O] = W3

    in_maps = []
    for c in range(NCORES):
        blk = np.empty((H, lay["ncols"]), np.float32)
        blk[:, lay["hj"]:lay["hj"] + NJ] = hj_t
        blk[:, lay["hib"]:lay["hib"] + IPC] = hib[c * IPC:(c + 1) * IPC].T
        blk[:, lay["w2"]:lay["w2"] + H] = W2
        blk[:, lay["b2"]] = b2
        blk[:, lay["w3"]:lay["w3"] + OPAD] = W3pad
        blk[:, lay["ones"]] = 1.0
        in_maps.append({"c": blk})

    nc = _get_nc(IPC, NJ)
    res = run_bass_kernel_spmd(nc, in_maps, list(range(NCORES)))
    out = np.concatenate(
        [res.results[c]["y"].reshape(IPC) for c in range(NCORES)]
    ).astype(np.float32)
    return out


def _numpy_ref(x, W1, b1, W2, b2, W3, b3):
    hi = x @ W1[:F]
    hj = x @ W1[F:]
    out = np.empty((N,), np.float32)
    for i in range(N):
        h = np.tanh(hi[i][None, :] + hj + b1[None, :])
        h = np.tanh(h @ W2 + b2[None, :])
        y = np.tanh(h @ W3 + b3[None, :])
        out[i] = y.sum()
    return out



# revision 6
# speedup vs baseline: 1.1675x; 1.1675x over previous
"""Trainium2 Bass kernel for nn_ConvPair (pairwise-MLP message passing).

Reference computation (N=1024 atoms, F=8 feats, H=128 hidden, O=3 out):
    hi = x @ W1[:F];  hj = x @ W1[F:]
    h  = tanh(hi[:,None,:] + hj[None,:,:] + b1)        # [N,N,H]
    h  = tanh(h @ W2 + b2)                             # [N,N,H]
    y  = tanh(h @ W3 + b3)                             # [N,N,O]
    out = y.sum(axis=(1,2))                            # [N]

Sharding: outer atom dim i split across 8 cores (128 i per core); the small
weights and the full atom table are replicated. No cross-core reduction.

The ACT (scalar) engine is the roofline: 2 tanh evaluations per (pair,
hidden) element = 262k columns/core at 1 col/cycle @1.2GHz ≈ 220us. All
matmuls run in bf16 (1 cyc/row on PE + fast weight load), keeping the PE
well under ACT, and the broadcast-add runs on the otherwise-idle DVE in a
2-byte packed perf mode. Final accuracy ~3e-3 rel (gate 2e-2).

Per-core device pipeline, hidden-major [H=128 partitions, ...]:
  add:   DVE  h1_pre[:,k,:] = HJ + hib_i          (bf16, per i)
  tanh1: ACT  one in-place tanh per group of G=16 i   [128, G*1024]
  mm1:   PE   W2 stationary (bf16, FWL), h1 moving -> ps1 [128,1024] f32
  tanh2: ACT  tanh(ps1 + b2) -> h2 bf16 SBUF      bias = b2 column
  mm3:   PE   8x (h2 128-pair chunk stationary) @ W3pad -> ps3 [128,4]
  tanh3: ACT  one in-place tanh per group over ps3 [128, G*32]
  red:   DVE  free-axis reduce ps3 -> ACC[:, i]   (j-offset partials)
  final: PE   ACC.T @ ones -> per-i scalars, ACT copy, DMA out.

The PE stream is software-pipelined (mm1 of step k+1 emitted before mm3 of
step k) so the PE ping-pong never blocks ACT; mm3/tanh3/reduce of a group's
last step drain during the next group's big tanh1.

b1 is folded into hib host-side; b2 via the ACT bias port; b3 is zeros for
this problem (asserted; a numpy fallback handles the hypothetical nonzero
case exactly).

Wait-discipline: walrus's datapath codegen supports only one semaphore
wait per instruction; _legalize_waits hoists extras onto chained NoOps.
"""

import json

import numpy as np
from contextlib import ExitStack

import bass_rust
import concourse.bass as bass
import concourse.tile as tile
from concourse import mybir
from concourse.bass_utils import run_bass_kernel_spmd

f32 = mybir.dt.float32
bf16 = mybir.dt.bfloat16
Tanh = mybir.ActivationFunctionType.Tanh

N, F, H, O = 1024, 8, 128, 3
NCORES = 8
IPC = N // NCORES  # 128 atoms (i) per core
NJ = N             # full j dimension on every core
G = 16             # i's per group (tanh1/tanh3 batching)
NCH = NJ // H      # 8 stage-3 chunks of 128 pairs
OPAD = 4           # W3 padded 3 -> 4 cols (aligned psum writes; pad col = 0)

# fp32-column layout of the packed per-core constant block [H, CCOLS]
_HJB, _HIB, _W2B, _W3B, _B2, _ONES = 0, 512, 640, 704, 706, 707
CCOLS = 708


def _bf16_pack(a):
    """[P, C] f32 -> [P, C//2] f32 whose words hold bf16 pairs (RNE)."""
    u = np.ascontiguousarray(a, np.float32).view(np.uint32)
    r = ((u >> 16) & 1) + 0x7FFF
    h = ((u + r) >> 16).astype(np.uint32)          # bf16 bit patterns
    h = h.reshape(a.shape[0], -1, 2)
    return (h[:, :, 0] | (h[:, :, 1] << 16)).view(np.float32)


# TPB instructions have a single 8-byte events field: 2 sync commands max
# (walrus rejects more).  Queue-engine DMA ops handle their own sync.
_MULTIWAIT_OK = {"DMACopy", "TriggeredCopy", "Call", "ISA"}


def _legalize_waits(nc):
    """Hoist excess semaphore waits from datapath instructions onto chained
    NoOps (one wait each) so every instruction fits walrus's sync budget."""
    j = json.loads(bass_rust.module_to_json_string(nc.m))
    counter = [0]

    def fix_list(insts):
        out = []
        for inst in insts:
            si = inst.get("sync_info")
            waits = (si or {}).get("on_wait", [])
            if si and len(waits) > 1 and inst.get("opcode") not in _MULTIWAIT_OK:
                # keep zero waits on the instruction; one NoOp per wait
                for w in waits:
                    counter[0] += 1
                    out.append({
                        "debug": inst.get("debug", 0),
                        "engine": inst["engine"],
                        "ins": [],
                        "outs": [],
                        "name": f"W-hoist-{counter[0]}",
                        "opcode": "NoOp",
                        "sync_info": {"on_update": [], "on_wait": [w]},
                    })
                si["on_wait"] = []
            out.append(inst)
        return out

    def walk(o):
        if isinstance(o, dict):
            if "instructions" in o and isinstance(o["instructions"], list):
                o["instructions"] = fix_list(o["instructions"])
            for v in o.values():
                walk(v)
        elif isinstance(o, list):
            for v in o:
                walk(v)

    walk(j)
    nc.m = bass_rust.module_from_json_string(json.dumps(j))
    return counter[0]


def _build(ipc, nj, reps=1):
    """Build the per-core Bass program (SPMD: same program, per-core data).

    reps > 1 repeats the main loop (recomputing identical results) and is
    used only for differential timing; outputs are unchanged."""
    assert nj % H == 0 and ipc % G == 0
    ngrp = ipc // G

    nc = bass.Bass()
    cparam = nc.declare_dram_parameter("c", [H, CCOLS], f32, isOutput=False)
    yparam = nc.declare_dram_parameter("y", [ipc, 1], f32, isOutput=True)

    with tile.TileContext(nc) as tc:
        with ExitStack() as ctx:
            consts = ctx.enter_context(tc.tile_pool(name="consts", bufs=1))
            h1p = ctx.enter_context(tc.tile_pool(name="h1p", bufs=2))
            h2p = ctx.enter_context(tc.tile_pool(name="h2p", bufs=3))
            scrp = ctx.enter_context(tc.tile_pool(name="scrp", bufs=1))
            accp = ctx.enter_context(tc.tile_pool(name="accp", bufs=1))
            # PSUM budget (8 banks): ps1 2x2 + ps3 2x1 + warm 1 + fin 1
            psA = ctx.enter_context(tc.tile_pool(name="psA", bufs=2, space="PSUM"))
            psB = ctx.enter_context(tc.tile_pool(name="psB", bufs=2, space="PSUM"))
            psW = ctx.enter_context(tc.tile_pool(name="psW", bufs=1, space="PSUM"))
            psF = ctx.enter_context(tc.tile_pool(name="psF", bufs=1, space="PSUM"))

            C = consts.tile([H, CCOLS], f32)
            nc.sync.dma_start(out=C, in_=cparam[:, :])

            HJB = C[:, _HJB:_HIB].bitcast(bf16)       # [H, nj]    bf16
            HIB = C[:, _HIB:_W2B]                     # [H, ipc]   f32
            W2B = C[:, _W2B:_W3B].bitcast(bf16)       # [H, H]     bf16
            W3B = C[:, _W3B:_B2].bitcast(bf16)        # [H, OPAD]  bf16
            B2 = C[:, _B2:_B2 + 1]                    # [H, 1]     f32
            ONES = C[:, _ONES:_ONES + 1]              # [H, 1]     f32

            ACC = accp.tile([H, ipc], f32)            # [j-offset, i] partials
            warm = scrp.tile([H, 1], f32, tag="warm")

            # --- warmups: let ACT, PE and DVE observe the const-DMA
            # semaphore (and load the tanh table) on single-wait
            # instructions, so no in-loop instruction waits on the DMA.
            nc.scalar.activation(out=warm, in_=B2, func=Tanh)
            warm_ps = psW.tile([1, 1], f32)
            nc.tensor.matmul(warm_ps, C[:, _B2:_B2 + 1], C[:, _B2:_B2 + 1],
                             start=True, stop=True)
            warmv = scrp.tile([H, 1], f32, tag="warmv")
            nc.vector.tensor_scalar_add(warmv, B2, 0.0)

            def flush_mm3(p):
                """Emit the deferred stage-3 matmuls for one (group, k)."""
                ps3, k, h2 = p
                for c in range(NCH):
                    nc.tensor.matmul(
                        ps3[:, k, c, :],
                        h2[:, c * H:(c + 1) * H],
                        W3B,
                        start=True, stop=True,
                    )

            def finish_group(g, ps3):
                """tanh3 + free-axis reduce into ACC for a completed group."""
                nc.scalar.activation(out=ps3[:, :, :, :], in_=ps3[:, :, :, :],
                                     func=Tanh)
                nc.vector.tensor_reduce(
                    out=ACC[:, g * G:(g + 1) * G],
                    in_=ps3.rearrange("p g c o -> p g (c o)"),
                    axis=mybir.AxisListType.X,
                    op=mybir.AluOpType.add,
                )

            def group_pass():
                pending = None        # deferred mm3 for the previous step
                closing = None        # (g, ps3) awaiting tanh3+reduce
                for g in range(ngrp):
                    # DVE: bf16 broadcast-adds; ACT: one big tanh per group
                    h1 = h1p.tile([H, G, nj], bf16)
                    for k in range(G):
                        i = g * G + k
                        nc.vector.tensor_scalar_add(
                            h1[:, k, :], HJB, HIB[:, i:i + 1])
                    nc.scalar.activation(out=h1[:, :, :], in_=h1[:, :, :],
                                         func=Tanh)
                    # drain the previous group's tail while tanh1 runs
                    if pending is not None:
                        flush_mm3(pending)
                        pending = None
                    if closing is not None:
                        finish_group(*closing)
                        closing = None

                    ps3 = psB.tile([H, G, NCH, OPAD], f32, tag="s3")
                    for k in range(G):
                        ps1 = psA.tile([H, nj], f32)
                        for t in range(nj // 512):
                            nc.tensor.matmul(
                                ps1[:, t * 512:(t + 1) * 512],
                                W2B, h1[:, k, t * 512:(t + 1) * 512],
                                start=True, stop=True)
                        if pending is not None:
                            flush_mm3(pending)
                        h2 = h2p.tile([H, nj], bf16)
                        nc.scalar.activation(out=h2, in_=ps1, func=Tanh,
                                             bias=B2)
                        pending = (ps3, k, h2)
                    closing = (g, ps3)
                flush_mm3(pending)
                finish_group(*closing)

            if reps == 1:
                group_pass()
            else:
                # hardware loop: identical iterations, used for timing only
                with tc.For_i(0, reps):
                    group_pass()

            # --- reduce over the 128 j-offset partitions: out = ACC.T @ ones
            fin = psF.tile([ipc, 1], f32)
            nc.tensor.matmul(fin, ACC, ONES, start=True, stop=True)
            yout = scrp.tile([ipc, 1], f32, tag="yout")
            nc.scalar.copy(yout, fin)
            nc.sync.dma_start(out=yparam[:, :], in_=yout)

    _legalize_waits(nc)
    return nc


_NC_CACHE = {}


def _get_nc(ipc, nj):
    key = (ipc, nj)
    if key not in _NC_CACHE:
        _NC_CACHE[key] = _build(ipc, nj)
    return _NC_CACHE[key]


def make_in_maps(x, W1, b1, W2, b2, W3, b3):
    """Per-core packed const blocks. Returns list of {"c": [H,CCOLS] f32}."""
    x = np.asarray(x, np.float32)
    W1 = np.asarray(W1, np.float32)
    hi = x @ W1[:F]          # [N, H]
    hj = x @ W1[F:]          # [N, H]
    hib = hi + np.asarray(b1, np.float32)[None, :]   # fold b1
    hj_t = np.ascontiguousarray(hj[:NJ].T)           # [H, nj]
    W3pad = np.zeros((H, OPAD), np.float32)
    W3pad[:, :O] = np.asarray(W3, np.float32)

    hjb = _bf16_pack(hj_t)
    w2b = _bf16_pack(np.asarray(W2, np.float32))
    w3b = _bf16_pack(W3pad)

    in_maps = []
    for c in range(NCORES):
        blk = np.empty((H, CCOLS), np.float32)
        blk[:, _HJB:_HIB] = hjb
        blk[:, _HIB:_W2B] = hib[c * IPC:(c + 1) * IPC].T
        blk[:, _W2B:_W3B] = w2b
        blk[:, _W3B:_B2] = w3b
        blk[:, _B2] = np.asarray(b2, np.float32)
        blk[:, _ONES] = 1.0
        in_maps.append({"c": blk})
    return in_maps


def kernel(x, W1, b1, W2, b2, W3, b3):
    b3 = np.asarray(b3, np.float32)
    if np.any(b3 != 0.0):
        # Never hit for this problem (spec fills b3 with zeros); exact
        # numpy fallback keeps the kernel correct for arbitrary inputs.
        return _numpy_ref(np.asarray(x, np.float32), np.asarray(W1, np.float32),
                          np.asarray(b1, np.float32), np.asarray(W2, np.float32),
                          np.asarray(b2, np.float32), np.asarray(W3, np.float32),
                          b3)

    in_maps = make_in_maps(x, W1, b1, W2, b2, W3, b3)
    nc = _get_nc(IPC, NJ)
    res = run_bass_kernel_spmd(nc, in_maps, list(range(NCORES)))
    out = np.concatenate(
        [res.results[c]["y"].reshape(IPC) for c in range(NCORES)]
    ).astype(np.float32)
    return out


def _numpy_ref(x, W1, b1, W2, b2, W3, b3):
    hi = x @ W1[:F]
    hj = x @ W1[F:]
    out = np.empty((N,), np.float32)
    for i in range(N):
        h = np.tanh(hi[i][None, :] + hj + b1[None, :])
        h = np.tanh(h @ W2 + b2[None, :])
        y = np.tanh(h @ W3 + b3[None, :])
        out[i] = y.sum()
    return out
